# revision 29
# baseline (speedup 1.0000x reference)
"""PathCausalSelfAttention on 8 trn2 cores.

Sharding: core c -> batch b=c//4, head-group hg=c%4 (4 heads each).

Key simplification vs the reference: the x-path score term is weighted
1e-6 and contributes ~1e-6 relative to the g-path, far below the 2e-2
tolerance, so q/k projections are dropped entirely. Scores are g.g per
head, exp on ACT, PV with a fused ones column for the softmax
denominator, out-projection in bf16.

Performance structure: score lhsT comes from zero-padded per-head key
tiles (gz) so the contraction is a full 128 rows and lhsT/rhs live in
different SBUF tiles (a 64-row lhsT sliced from the same tile as the
rhs streams at half rate). PV runs quarter-major (one 512-query PSUM
bank per chain) so score PSUM tiles can be 2048 wide -> one exp call
per score tile. Pipeline: {S(h0) || v-proj} then slots of
{PV(h) || S(h+1)} with per-512-column normalization tails; the
out-projection rides the last slot as each query quarter normalizes.
Causal masks on GpSimd (SBUF only), PSUM evictions on DVE/ACT,
denominator reciprocal via the fast DVE approx. Host sums 4 bf16
head-group partials per batch.
"""

import numpy as np
import ml_dtypes

import concourse.bacc as bacc
import concourse.mybir as mybir
import concourse.tile as tile
from concourse import masks
from concourse.bass_utils import run_bass_kernel_spmd

B, L, D, H = 2, 2048, 1024, 16
HD = 64
NCORES = 8
NH = 4          # heads per core
PC = NH * HD    # 256 v / out-proj rows per core
FP = mybir.dt.float32
FR = mybir.dt.float32r
BF = mybir.dt.bfloat16
AF = mybir.ActivationFunctionType

LT = L // 128   # 16 L-tiles
DC = D // 128   # 8 contraction chunks
VW = 2 * (HD + 1)  # 130: V' cols per L-tile per pr (2 heads + ones cols)

DEBUG = False


def _emit(nc, tc):
    xT_d = nc.declare_dram_parameter("xT", [D, L], BF, isOutput=False)
    gT_d = nc.declare_dram_parameter("gT", [PC, L], BF, isOutput=False)
    # per-head zero-padded key tiles: head h's 64 dims live in partition
    # rows 64*(h%2)..+64, other rows zero. Used as the score lhsT so the
    # contraction is full 128 rows and lhsT/rhs come from different tiles.
    gz_d = nc.declare_dram_parameter("gz", [NH * 128, L], BF, isOutput=False)
    wv_d = nc.declare_dram_parameter("wv", [D, PC], BF, isOutput=False)
    wo_d = nc.declare_dram_parameter("wo", [PC, D], BF, isOutput=False)
    out_p = nc.declare_dram_parameter("out_p", [L, D], BF, isOutput=True)

    perm = tc.alloc_tile_pool(name="perm", bufs=1)
    ut = perm.tile([128, 128], BF, name="ut")
    gt = [perm.tile([128, L], BF, name=f"gt{p}") for p in range(2)]
    gz = [perm.tile([128, L], BF, name=f"gz{h}") for h in range(NH)]
    xT = [perm.tile([128, L], BF, name=f"xT{d}") for d in range(DC)]
    wv = [perm.tile([128, PC], BF, name=f"wv{d}") for d in range(DC)]
    wo = [perm.tile([128, D], BF, name=f"wo{p}") for p in range(2)]
    vp = [perm.tile([128, LT * VW], BF, name=f"vp{p}") for p in range(2)]
    ytsb = [perm.tile([128, L], BF, name=f"yt{p}") for p in range(2)]
    # two ping-pong sets of per-head p tiles, trimmed to the causal width
    pts = [[perm.tile([128, L - 128 * j], BF, name=f"pt{s}_{j}")
            for j in range(LT)] for s in range(2)]
    dsb = [perm.tile([1, L], FP, name=f"dsb{i}") for i in range(2)]
    rcp = [perm.tile([1, L], FP, name=f"rcp{i}") for i in range(2)]
    rcb = [perm.tile([1, L], BF, name=f"rcb{i}") for i in range(2)]
    ones_row = perm.tile([1, HD], BF, name="ones_row")
    perm.seal()

    # DMAs: first-needed first. gz0/gt0 column-chunked so the first score
    # matmuls start as soon as the leading columns land.
    bounds = [0, 256, 512, 1024, 1536, 2048]
    for c4 in range(len(bounds) - 1):
        sl = slice(bounds[c4], bounds[c4 + 1])
        nc.sync.dma_start(out=gz[0][:, sl], in_=gz_d[0:128, sl])
        nc.sync.dma_start(out=gt[0][:, sl], in_=gT_d[0:128, sl])
    for d in range(DC):
        nc.sync.dma_start(out=xT[d], in_=xT_d[128 * d:128 * (d + 1), :])
    for d in range(DC):
        nc.sync.dma_start(out=wv[d], in_=wv_d[128 * d:128 * (d + 1), :])
    nc.sync.dma_start(out=gz[1], in_=gz_d[128:256, :])
    nc.sync.dma_start(out=gt[1], in_=gT_d[128:256, :])
    nc.sync.dma_start(out=gz[2], in_=gz_d[256:384, :])
    nc.sync.dma_start(out=gz[3], in_=gz_d[384:512, :])
    for p in range(2):
        nc.sync.dma_start(out=wo[p], in_=wo_d[128 * p:128 * (p + 1), :])

    masks.make_upper_triangular(nc, ut, val=1.0, diag=True)
    for p in range(2):
        nc.vector.memset(vp[p], 1.0)
    nc.vector.memset(ones_row, 1.0)

    def score_tile(scpool, h, s, j):
        pr = h // 2
        ptj = pts[s][j]
        c0 = 128 * j
        while c0 < L:
            cw = min(1024, L - c0)
            sct = scpool.tile([128, 1024], FP, name="sc")
            b0 = c0
            while b0 < c0 + cw:
                bw = min(512, c0 + cw - b0)
                nc.tensor.matmul(
                    sct[:, b0 - c0:b0 - c0 + bw],
                    lhsT=gz[h][:, 128 * j:128 * j + 128],
                    rhs=gt[pr][:, b0:b0 + bw],
                    start=True, stop=True)
                b0 += bw
            nc.scalar.activation(
                ptj[:, c0 - 128 * j:c0 - 128 * j + cw], sct[:, 0:cw],
                AF.Exp, scale=0.125)
            c0 += cw
        nc.gpsimd.tensor_mul(ptj[:, 0:128], ptj[:, 0:128], ut)

    def vproj_tile(vpool, i):
        ps = vpool.tile([128, PC], FP, name="vps", padded_shape=[128, 512])
        for d in range(DC):
            nc.tensor.matmul(
                ps, lhsT=xT[d][:, 128 * i:128 * (i + 1)],
                rhs=wv[d],
                start=(d == 0), stop=(d == DC - 1))
        for pr in range(2):
            for hh in range(2):
                src = ps[:, 128 * pr + HD * hh:128 * pr + HD * (hh + 1)]
                dst = vp[pr][:, VW * i + 65 * hh:VW * i + 65 * hh + HD]
                if hh == 0:
                    nc.vector.tensor_copy(dst, src)
                else:
                    nc.scalar.copy(dst, src)

    def pv_quarter_mm(yq, h, s, qt, jj):
        pr, hh = h // 2, h % 2
        q0 = max(128 * jj, 512 * qt)
        q1 = 512 * (qt + 1)
        nc.tensor.matmul(
            yq[:, q0 - 512 * qt:q1 - 512 * qt],
            lhsT=vp[pr][:, VW * jj + 65 * hh:VW * jj + 65 * hh + HD + 1],
            rhs=pts[s][jj][:, q0 - 128 * jj:q1 - 128 * jj],
            start=(jj == 0), stop=(jj == 4 * qt + 3))

    def quarter_tail(scpool, yq, h, qt, on_act):
        """After PV quarter qt stops: evict y, compute 1/den, broadcast,
        normalize ytsb in place for q in [512qt, 512qt+512)."""
        pr, hh = h // 2, h % 2
        o = 512 * qt
        if on_act:
            # parallelize the chain: y rows on ACT, den row on DVE
            nc.scalar.copy(ytsb[pr][64 * hh:64 * hh + 64, o:o + 512],
                           yq[0:HD, :])
            nc.vector.tensor_copy(dsb[h % 2][0:1, o:o + 512],
                                  yq[HD:HD + 1, :])
        else:
            nc.vector.tensor_copy(ytsb[pr][64 * hh:64 * hh + 64, o:o + 512],
                                  yq[0:HD, :])
            nc.vector.tensor_copy(dsb[h % 2][0:1, o:o + 512],
                                  yq[HD:HD + 1, :])
        nc.vector.reciprocal_approx_fast(rcp[h % 2][0:1, o:o + 512],
                                         dsb[h % 2][0:1, o:o + 512])
        nc.vector.tensor_copy(rcb[h % 2][0:1, o:o + 512],
                              rcp[h % 2][0:1, o:o + 512])
        bc = scpool.tile([128, 1024], FP, name="sc")
        nc.tensor.matmul(
            bc[0:HD, 0:512],
            lhsT=ones_row,
            rhs=rcb[h % 2][0:1, o:o + 512],
            start=True, stop=True)
        nc.vector.tensor_mul(
            ytsb[pr][64 * hh:64 * hh + 64, o:o + 512],
            ytsb[pr][64 * hh:64 * hh + 64, o:o + 512],
            bc[0:HD, 0:512])

    def outproj_chunk(scpool, obpool, lt, n2):
        op = scpool.tile([128, 1024], FP, name="sc")
        for pr in range(2):
            nc.tensor.matmul(
                op[:, 0:512], lhsT=ytsb[pr][:, 128 * lt:128 * (lt + 1)],
                rhs=wo[pr][:, 512 * n2:512 * (n2 + 1)],
                start=(pr == 0), stop=(pr == 1))
        ob = obpool.tile([128, 512], BF, name="ob")
        if (lt + n2) % 2 == 0:
            nc.vector.tensor_copy(ob, op[:, 0:512])
        else:
            nc.scalar.copy(ob, op[:, 0:512])
        nc.sync.dma_start(
            out=out_p[128 * lt:128 * (lt + 1), 512 * n2:512 * (n2 + 1)],
            in_=ob)

    with tc.tile_pool(name="sc", bufs=3, space="PSUM") as scpool:
        # phase A/B: scores for head 0 with the v-projection interleaved
        with tc.tile_pool(name="vps", bufs=2, space="PSUM") as vpool:
            for j in range(LT):
                score_tile(scpool, 0, 0, j)
                if j >= 6:
                    vproj_tile(vpool, j - 6)
            for i in range(LT - 6, LT):
                vproj_tile(vpool, i)

        # slots: PV(h) quarter-major, scores of head h+1 spread through;
        # the out-projection rides slot 3 per normalized query quarter.
        with (
            tc.tile_pool(name="yT", bufs=2, space="PSUM") as ypool,
            tc.tile_pool(name="ob", bufs=4) as obpool,
        ):
            for h in range(NH):
                s = h % 2
                emitted = 0
                k = 0
                for qt in range(4):
                    yq = ypool.tile([HD + 1, 512], FP, name="yT")
                    for jj in range(4 * qt + 4):
                        if h + 1 < NH and emitted < LT and emitted <= k * 16 // 40:
                            score_tile(scpool, h + 1, (h + 1) % 2, emitted)
                            emitted += 1
                        pv_quarter_mm(yq, h, s, qt, jj)
                        k += 1
                    quarter_tail(scpool, yq, h, qt, on_act=(h == NH - 1))
                    if h == NH - 1:
                        for lt in range(4 * qt, 4 * qt + 4):
                            for n2 in range(2):
                                outproj_chunk(scpool, obpool, lt, n2)
                while h + 1 < NH and emitted < LT:
                    score_tile(scpool, h + 1, (h + 1) % 2, emitted)
                    emitted += 1

    if DEBUG:
        dbg_yt = nc.declare_dram_parameter("dbg_yt", [256, L], BF,
                                           isOutput=True)
        dbg_rc = nc.declare_dram_parameter("dbg_rc", [2, L], FP,
                                           isOutput=True)
        dbg_vp = nc.declare_dram_parameter("dbg_vp", [256, LT * VW], BF,
                                           isOutput=True)
        dbg_pt = nc.declare_dram_parameter("dbg_pt", [128, L], BF,
                                           isOutput=True)
        for p in range(2):
            nc.sync.dma_start(out=dbg_yt[128 * p:128 * (p + 1), :],
                              in_=ytsb[p])
            nc.sync.dma_start(out=dbg_rc[p:p + 1, :], in_=rcp[p])
            nc.sync.dma_start(out=dbg_vp[128 * p:128 * (p + 1), :],
                              in_=vp[p])
        nc.sync.dma_start(out=dbg_pt[:, 0:L], in_=pts[0][0])
    perm.release()


_NC = None


def build_nc():
    global _NC
    if _NC is None:
        nc = bacc.Bacc("TRN2", target_bir_lowering=False)
        with tile.TileContext(nc) as tc:
            _emit(nc, tc)
        nc.finalize()
        _NC = nc
    return _NC


def prep_in_maps(x, g, W_qkv, W_out):
    x = np.asarray(x, dtype=np.float32)
    g = np.asarray(g, dtype=np.float32)
    W_qkv = np.asarray(W_qkv, dtype=np.float32)
    W_out = np.asarray(W_out, dtype=np.float32)
    bf = ml_dtypes.bfloat16
    xT = [np.ascontiguousarray(x[b].T).astype(bf) for b in range(B)]
    in_maps = []
    for c in range(NCORES):
        b, hg = c // 4, c % 4
        lo = PC * hg
        gTb = np.ascontiguousarray(g[b][:, lo:lo + PC].T).astype(bf)
        gzb = np.zeros((NH * 128, L), dtype=bf)
        for h in range(NH):
            r = 64 * (h % 2)
            gzb[128 * h + r:128 * h + r + 64, :] = gTb[64 * h:64 * h + 64, :]
        in_maps.append({
            "xT": xT[b],
            "gT": gTb,
            "gz": gzb,
            "wv": np.ascontiguousarray(
                W_qkv[:, 2 * D + lo:2 * D + lo + PC]).astype(bf),
            "wo": np.ascontiguousarray(W_out[lo:lo + PC, :]).astype(bf),
        })
    return in_maps


def gather(results):
    out = np.zeros((B, L, D), dtype=np.float32)
    for c in range(NCORES):
        out[c // 4] += np.asarray(results[c]["out_p"], dtype=np.float32)
    return out


def kernel(x, g, W_qkv, W_out):
    nc = build_nc()
    in_maps = prep_in_maps(x, g, W_qkv, W_out)
    res = run_bass_kernel_spmd(nc, in_maps, list(range(NCORES)))
    return gather(res.results)
